# revision 64
# baseline (speedup 1.0000x reference)
"""GQA attention kernel for 8 Trainium2 NeuronCores (tensor-parallel over heads).

Self-contained: hardcodes shapes from the problem spec.
  x  [2, 1024, 4096]  Wq [4096, 4096]  Wk/Wv [4096, 1024]  Wo [4096, 4096]  bo [4096]
  32 q heads, 8 KV groups, head dim 128, RoPE theta 10000, causal softmax.

Sharding (Megatron-style TP): core c owns KV group c and query heads 4c..4c+3.
  - x replicated to every core (no input collective); Wq/Wk/Wv column-sharded
    per head group (RoPE-permuted, score scale folded into Wq)
  - attention computed locally per core; attnT AllGathered per batch
    (issued mid-stream so the gathers overlap the next batch's attention
    and the first output-projection tiles)
  - Wo column-sharded -> each core returns its 512 output columns; host
    concatenates and adds bo.

Pipeline notes (sim-guided): stage-2's first rope/v loads are issued from
inside stage 1 (data ready after its second t-chunk); Wo prefetch trickles
between batch-0 head loads; psum evictions run on DVE and the causal mask
on Pool so the ACT engine only runs exp (avoids activation-table reloads);
all inter-stage DMAs are batched to respect the ~625ns/issue HWDGE rate.
"""

import numpy as np
import ml_dtypes

import concourse.bass as bass
import concourse.mybir as mybir
import concourse.tile as tile
from concourse import bacc
from concourse import bass_utils

N_CORES = 8
B, T, C = 2, 1024, 4096
H, G, D = 32, 8, 128
REP = H // G            # q heads per KV group/core = 4
BT = B * T              # 2048
HD_SHARD = REP * D      # 512 q columns per core
CO_SHARD = C // N_CORES  # 512 output columns per core
ROPE_THETA = 10000.0

F32 = mybir.dt.float32
F32R = mybir.dt.float32r
F16 = mybir.dt.float16

_CACHE = {}


def _balanced_chunks(total, maxc=512, quantum=128):
    """Split `total` (multiple of quantum) into chunks <= maxc, balanced."""
    nblk = total // quantum
    n = -(-total // maxc)
    sizes = []
    for i in range(n):
        take = nblk // n + (1 if i < nblk % n else 0)
        sizes.append(take * quantum)
    return sizes


def _build_nc(iters=1, single_core_sim=False):
    # single_core_sim: build a 1-core, collective-free twin (AllGathers become
    # local DMA stand-ins) for TimelineSim-based attribution. Graded path uses
    # the default.
    ndev = 1 if single_core_sim else N_CORES
    nc = bacc.Bacc("TRN2", target_bir_lowering=False, debug=False, num_devices=ndev)

    # ---- I/O ----
    # x is replicated to every core (standard TP: activations replicated,
    # weights head-sharded) — no input AllGather needed.
    xT_full = nc.dram_tensor("xT_full", [C, BT], F16, kind="ExternalInput")
    wq = nc.dram_tensor("wq", [C, HD_SHARD], F16, kind="ExternalInput")
    wk = nc.dram_tensor("wk", [C, D], F16, kind="ExternalInput")
    wv = nc.dram_tensor("wv", [C, D], F16, kind="ExternalInput")
    wo = nc.dram_tensor("wo", [H * D, CO_SHARD], F16, kind="ExternalInput")
    cos2 = nc.dram_tensor("cos2", [D, T], F16, kind="ExternalInput")
    sinpm = nc.dram_tensor("sinpm", [D, T], F16, kind="ExternalInput")
    tri = nc.dram_tensor("tri", [128, 128], F16, kind="ExternalInput")
    idin = nc.dram_tensor("idin", [128, 128], F16, kind="ExternalInput")
    out = nc.dram_tensor("out", [BT, CO_SHARD], F16, kind="ExternalOutput")

    # ---- DRAM scratch ----
    # qT/kT rows: q heads (4*128) then k head (128). consumed by DVE rope (f16).
    qkT_dram = nc.dram_tensor("qkT_dram", [(REP + 1) * D, BT], F16, kind="Internal")
    vT_dram = nc.dram_tensor("vT_dram", [D, BT], F16, kind="Internal")
    TG = 512
    NTG = BT // TG  # 4
    attnT_dram = [
        nc.dram_tensor(f"attnT_dram_{g}", [HD_SHARD, T], F16, kind="Internal")
        for g in range(B)
    ]
    attnT_full = [
        nc.dram_tensor(
            f"attnT_full_{g}", [H * D, T], F16, kind="Internal", addr_space="Shared"
        )
        for g in range(B)
    ]

    with tile.TileContext(nc) as tc:
      for _it in range(iters):
        # ================= Stage 1: QKV projection =================
        # qkvT[m, t] = sum_c W[c, m] * xT[c, t];  m = q(512) | k(128) | v(128)
        M_ALL = HD_SHARD + 2 * D  # 768
        M_CHUNKS = M_ALL // 128   # 6 (0-3 q heads, 4 k, 5 v)
        KSUB = C // 128           # 32
        NT = 512                  # t-cols per chunk
        xT_r = xT_full[:].rearrange("(ko p) t -> p ko t", p=128)
        wq_r = wq[:].rearrange("(ko p) m -> p ko m", p=128)
        wk_r = wk[:].rearrange("(ko p) m -> p ko m", p=128)
        wv_r = wv[:].rearrange("(ko p) m -> p ko m", p=128)

        qkT_r = qkT_dram[:].rearrange("(m p) t -> p m t", p=128)
        wo_r = wo[:].rearrange("(ko p) m -> p ko m", p=128)
        NCH = T // 128  # 8 chunks of both s and t

        # s3_w outermost (LIFO pool stack): closed after stage 3
        s3_w_ctx = tc.tile_pool(name="s3_w", bufs=1)
        s3_w = s3_w_ctx.__enter__()
        wo_sb = s3_w.tile([128, (H * D) // 128, CO_SHARD], F16)
        with (
            tc.tile_pool(name="s2_const", bufs=1) as s2_const,
            tc.tile_pool(name="s2_kv", bufs=2) as s2_kv,
            tc.tile_pool(name="s2_q", bufs=3) as s2_q,
            tc.tile_pool(name="s2_tmp", bufs=5) as s2_tmp,
        ):
            ident = s2_const.tile([128, 128], F16)
            tri_sb = s2_const.tile([128, 128], F16)
            cos_sb = s2_const.tile([128, T], F16)
            sin_sb = s2_const.tile([128, T], F16)

            def issue_rope_loads(dram_rows, tcol):
                """Queue the src + swapped-half loads for one head."""
                r0 = dram_rows.start
                src = s2_tmp.tile([128, T], F16, tag="rope_src")
                nc.sync.dma_start(src[:], qkT_dram[dram_rows, tcol])
                swp = s2_tmp.tile([128, T], F16, tag="rope_swp")
                nc.sync.dma_start(swp[0:64], qkT_dram[r0 + 64:r0 + 128, tcol])
                nc.sync.dma_start(swp[64:128], qkT_dram[r0:r0 + 64, tcol])
                return src, swp

            def finish_rope(dst, src, swp):
                """dst = src * [cos;cos] + swap_halves(src) * [-sin;+sin]"""
                tmp = s2_tmp.tile([128, T], F16, tag="rope_tmp")
                nc.vector.tensor_tensor(tmp[:], swp[:], sin_sb[:], mybir.AluOpType.mult)
                nc.vector.tensor_tensor(dst[:], src[:], cos_sb[:], mybir.AluOpType.mult)
                nc.vector.tensor_tensor(dst[:], dst[:], tmp[:], mybir.AluOpType.add)

            pre = {}  # (kind, b) -> prefetched tiles, issued mid-stage-1

            with (
                tc.tile_pool(name="s1_w", bufs=1) as s1_w,
                tc.tile_pool(name="s1_x", bufs=4) as s1_x,
                tc.tile_pool(name="s1_ev", bufs=2) as s1_ev,
                tc.tile_pool(name="s1_psum", bufs=8, space="PSUM") as s1_psum,
            ):
                KQ = 8  # k-subtiles per x tile (quarter of KSUB)
                w_sb = s1_w.tile([128, KSUB, M_ALL], F16)
                # first x quarter queued before the weights: the first matmuls
                # need x(q0) + w(q0), everything else streams behind
                x_first = s1_x.tile([128, KQ, NT], F16, tag="s1x")
                nc.sync.dma_start(x_first[:], xT_r[:, 0:KQ, 0:NT])
                for kq in range(KSUB // KQ):
                    ks = slice(kq * KQ, (kq + 1) * KQ)
                    nc.sync.dma_start(w_sb[:, ks, 0:HD_SHARD], wq_r[:, ks, :])
                    nc.sync.dma_start(
                        w_sb[:, ks, HD_SHARD:HD_SHARD + D], wk_r[:, ks, :]
                    )
                    nc.sync.dma_start(
                        w_sb[:, ks, HD_SHARD + D:M_ALL], wv_r[:, ks, :]
                    )
                nc.sync.dma_start(ident[:], idin[:])
                nc.sync.dma_start(tri_sb[:], tri[:])
                nc.sync.dma_start(cos_sb[:], cos2[:])
                nc.sync.dma_start(sin_sb[:], sinpm[:])

                for n in range(BT // NT):  # 4 chunks of 512 t-cols
                    psums = [
                        s1_psum.tile([128, NT], F32, name=f"s1ps_{m}", tag="s1ps")
                        for m in range(M_CHUNKS)
                    ]
                    for kq in range(KSUB // KQ):
                        if n == 0 and kq == 0:
                            x_sb = x_first
                        else:
                            x_sb = s1_x.tile([128, KQ, NT], F16, tag="s1x")
                            nc.sync.dma_start(
                                x_sb[:],
                                xT_r[:, kq * KQ:(kq + 1) * KQ, n * NT:(n + 1) * NT],
                            )
                        for m in range(M_CHUNKS):
                            for k in range(KQ):
                                nc.tensor.matmul(
                                    psums[m][:],
                                    w_sb[:, kq * KQ + k, m * 128:(m + 1) * 128],
                                    x_sb[:, k, :],
                                    start=(kq == 0 and k == 0),
                                    stop=(kq == KSUB // KQ - 1 and k == KQ - 1),
                                )
                    # batched eviction: 5 qk psums -> one tile, one DMA
                    qk_ev = s1_ev.tile([128, M_CHUNKS - 1, NT], F16, tag="s1qk")
                    for m in range(M_CHUNKS - 1):
                        nc.vector.tensor_copy(qk_ev[:, m, :], psums[m][:])
                    nc.sync.dma_start(
                        qkT_r[:, :, n * NT:(n + 1) * NT], qk_ev[:]
                    )
                    v_ev = s1_ev.tile([128, NT], F16, tag="s1v")
                    nc.vector.tensor_copy(v_ev[:], psums[M_CHUNKS - 1][:])
                    nc.sync.dma_start(vT_dram[:, n * NT:(n + 1) * NT], v_ev[:])

                    if n == 1:
                        # batch 0's t-range is fully evicted: prefetch its
                        # k/v/first-q loads under the rest of stage 1
                        tc0 = slice(0, T)
                        pre["k0"] = issue_rope_loads(
                            slice(REP * D, (REP + 1) * D), tc0
                        )
                        vT_sb0 = s2_kv.tile([128, T], F16, tag="vT")
                        nc.sync.dma_start(vT_sb0[:], vT_dram[:, tc0])
                        pre["v0"] = vT_sb0
                        pre["q00"] = issue_rope_loads(slice(0, D), tc0)

        # ================= Stage 2: attention per (b, local head) =================
            # (s2 prologue pools stay open from above)
            KSUB3 = (H * D) // 128  # 32
            aT_rs = [
                attnT_full[g // 2][:, (g % 2) * TG:(g % 2 + 1) * TG]
                .rearrange("(ko p) t -> p ko t", p=128)
                for g in range(NTG)
            ]
            s3_a_ctx = tc.tile_pool(name="s3_a", bufs=4)
            s3_a = s3_a_ctx.__enter__()
            with (
                tc.tile_pool(name="s2_probs", bufs=2) as s2_probs,
                tc.tile_pool(name="s2_out", bufs=3) as s2_out,
                tc.tile_pool(name="s2_ps_sc", bufs=2, space="PSUM") as s2_ps_sc,
                tc.tile_pool(name="s2_ps_pv", bufs=2, space="PSUM") as s2_ps_pv,
                tc.tile_pool(name="s2_ps_tr", bufs=2, space="PSUM") as s2_ps_tr,
            ):
              for b in range(B):
                tcol = slice(b * T, (b + 1) * T)
                # k rope
                k_rope = s2_kv.tile([128, T], F16, tag="k_rope")
                finish_rope(k_rope, *pre[f"k{b}"])
                vT_sb = pre[f"v{b}"]
                v_sb = s2_kv.tile([128, NCH, D + 1], F16, tag="v_ext")
                nc.vector.memset(v_sb[:, :, D:D + 1], 1.0)
                for j in range(NCH):
                    ps_tr = s2_ps_tr.tile([128, 128], F16, tag="ps_tr")
                    nc.tensor.transpose(ps_tr[:], vT_sb[:, j * 128:(j + 1) * 128], ident[:])
                    nc.vector.tensor_copy(v_sb[:, j, 0:D], ps_tr[:])

                for h in range(REP):
                    q_rope = s2_q.tile([128, T], F16, tag="q_rope")
                    if h == 0:
                        finish_rope(q_rope, *pre[f"q0{b}"])
                    else:
                        finish_rope(
                            q_rope,
                            *issue_rope_loads(slice(h * D, (h + 1) * D), tcol),
                        )
                    if b == 0:
                        # Wo prefetch trickles in between per-head loads
                        nc.sync.dma_start(
                            wo_sb[:, h * 8:(h + 1) * 8, :],
                            wo_r[:, h * 8:(h + 1) * 8, :],
                        )
                    if b == 0 and h == 2:
                        # prefetch batch 1's k/v loads under batch 0's tail
                        tc1 = slice(T, 2 * T)
                        pre["k1"] = issue_rope_loads(
                            slice(REP * D, (REP + 1) * D), tc1
                        )
                        vT_sb1 = s2_kv.tile([128, T], F16, tag="vT")
                        nc.sync.dma_start(vT_sb1[:], vT_dram[:, tc1])
                        pre["v1"] = vT_sb1
                    if b == 0 and h == 3:
                        pre["q01"] = issue_rope_loads(slice(0, D), slice(T, 2 * T))

                    # scoresT[s, t] = k_rope.T @ q_rope, exp -> probs (bf16).
                    # One 2-bank psum strip per s-chunk: matmuls fill 512-col
                    # (bank-aligned) slices, then a single wide exp — fewer
                    # ACT instructions, same PE work. 512-alignment keeps each
                    # matmul's start=True pending-zero marks on its own banks.
                    probs = s2_probs.tile([128, NCH, T], F16, tag="probs")
                    for j in range(NCH):
                        t0 = j * 128
                        width = T - t0
                        ps_sc = s2_ps_sc.tile([128, 1024], F32, tag="ps_sc")
                        for off in range(0, width, 512):
                            w = min(512, width - off)
                            nc.tensor.matmul(
                                ps_sc[:, off:off + w],
                                k_rope[:, j * 128:(j + 1) * 128],
                                q_rope[:, t0 + off:t0 + off + w],
                                start=True,
                                stop=True,
                            )
                        nc.scalar.activation(
                            probs[:, j, t0:T],
                            ps_sc[:, 0:width],
                            mybir.ActivationFunctionType.Exp,
                        )
                        # causal mask on the diagonal block (s > t -> 0); Pool
                        # engine, keeping DVE free for rope
                        nc.gpsimd.tensor_tensor(
                            probs[:, j, t0:t0 + 128],
                            probs[:, j, t0:t0 + 128],
                            tri_sb[:],
                            mybir.AluOpType.mult,
                        )

                    # PV: out[t, d | sum] = probs.T @ [v | 1]
                    for i in range(NCH):
                        ps_pv = s2_ps_pv.tile([128, D + 1], F32, tag="ps_pv")
                        for j in range(i + 1):
                            nc.tensor.matmul(
                                ps_pv[:],
                                probs[:, j, i * 128:(i + 1) * 128],
                                v_sb[:, j, :],
                                start=(j == 0),
                                stop=(j == i),
                            )
                        rcp = s2_tmp.tile([128, 1], F32, tag="rcp")
                        nc.vector.reciprocal(rcp[:], ps_pv[:, D:D + 1])
                        attn_sb = s2_out.tile([128, D], F16, tag="attn")
                        nc.vector.tensor_scalar_mul(attn_sb[:], ps_pv[:, 0:D], rcp[:])
                        ps_tr2 = s2_ps_tr.tile([128, 128], F16, tag="ps_tr")
                        nc.tensor.transpose(ps_tr2[:], attn_sb[:], ident[:])
                        attnT_sb = s2_out.tile([128, 128], F16, tag="attnT")
                        nc.vector.tensor_copy(attnT_sb[:], ps_tr2[:])
                        nc.sync.dma_start(
                            attnT_dram[b][h * D:(h + 1) * D,
                                          i * 128:(i + 1) * 128],
                            attnT_sb[:],
                        )

                # AllGather this batch's attnT tgs as soon as they complete,
                # overlapping the next batch's attention / stage 3 compute.
                if single_core_sim:
                    nc.sync.dma_start(
                        attnT_full[b][0:HD_SHARD, :], attnT_dram[b][:]
                    )
                else:
                    nc.gpsimd.collective_compute(
                        "AllGather",
                        mybir.AluOpType.bypass,
                        replica_groups=[list(range(N_CORES))],
                        ins=[attnT_dram[b][:].opt()],
                        outs=[attnT_full[b][:].opt()],
                    )
                if b == 0:
                    # prefetch stage-3's first attnT tile under batch 1
                    for khalf in range(2):
                        a0 = s3_a.tile([128, KSUB3 // 2, TG], F16, tag="s3a")
                        nc.sync.dma_start(
                            a0[:], aT_rs[0][:, khalf * 16:(khalf + 1) * 16, :]
                        )
                        pre[("a0", khalf)] = a0

        # ================= Stage 3: output projection =================
        # out[t, co] = sum_hd attnT_full[hd, t] * wo[hd, co]
        # (nested in the prologue scope: pool stack stays LIFO, and the tg0
        # attnT tiles prefetched during stage 2 come from s3_a)
            with (
                tc.tile_pool(name="s3_ev", bufs=3) as s3_ev,
                tc.tile_pool(name="s3_psum", bufs=8, space="PSUM") as s3_psum,
            ):
                for tg in range(BT // TG):  # 4
                    psums3 = [
                        s3_psum.tile([128, CO_SHARD], F32, name=f"s3ps_{m}", tag="s3ps")
                        for m in range(TG // 128)
                    ]
                    for khalf in range(2):
                        if tg == 0:
                            a_sb = pre[("a0", khalf)]
                        else:
                            a_sb = s3_a.tile([128, KSUB3 // 2, TG], F16, tag="s3a")
                            nc.sync.dma_start(
                                a_sb[:],
                                aT_rs[tg][:, khalf * 16:(khalf + 1) * 16, :],
                            )
                        for m in range(TG // 128):
                            for k in range(KSUB3 // 2):
                                nc.tensor.matmul(
                                    psums3[m][:],
                                    a_sb[:, k, m * 128:(m + 1) * 128],
                                    wo_sb[:, khalf * 16 + k, :],
                                    start=(khalf == 0 and k == 0),
                                    stop=(khalf == 1 and k == KSUB3 // 2 - 1),
                                )
                    for m in range(TG // 128):
                        ev = s3_ev.tile([128, CO_SHARD], F16, tag="s3ev")
                        nc.vector.tensor_copy(ev[:], psums3[m][:])
                        nc.sync.dma_start(
                            out[tg * TG + m * 128:tg * TG + (m + 1) * 128, :], ev[:]
                        )
            s3_a_ctx.__exit__(None, None, None)
        s3_w_ctx.__exit__(None, None, None)

    nc.compile()
    return nc


def _rope_perm():
    """Column permutation within one head: [0,2,...,126, 1,3,...,127]."""
    return np.concatenate([np.arange(0, D, 2), np.arange(1, D, 2)])


def _host_prep(x, Wq, Wk, Wv, Wo, bo):
    x = np.asarray(x, dtype=np.float32)
    Wq = np.asarray(Wq, dtype=np.float32)
    Wk = np.asarray(Wk, dtype=np.float32)
    Wv = np.asarray(Wv, dtype=np.float32)
    Wo = np.asarray(Wo, dtype=np.float32)

    # cast to fp16 first (halves the bytes moved by the transpose copy)
    xT = np.ascontiguousarray(x.reshape(BT, C).astype(np.float16).T)

    perm = _rope_perm()
    scale = np.float32(D ** -0.5)

    # rope-permute all heads at once; fold the score scale into Wq.
    # perm == [evens | odds], done as reshape+transpose (faster than a gather)
    Wqp = np.ascontiguousarray(
        (Wq * scale).astype(np.float16)
        .reshape(C, H, D // 2, 2).transpose(0, 1, 3, 2).reshape(C, H, D))
    Wkp = np.ascontiguousarray(
        Wk.astype(np.float16)
        .reshape(C, G, D // 2, 2).transpose(0, 1, 3, 2).reshape(C, G, D))
    Wv16 = Wv.astype(np.float16).reshape(C, G, D)
    Wo16 = Wo.astype(np.float16)

    freqs = 1.0 / (ROPE_THETA ** (np.arange(0, D, 2, dtype=np.float64) / D))
    angle = np.arange(T, dtype=np.float64)[:, None] * freqs[None, :]  # [T, 64]
    cosh = np.cos(angle).T.astype(np.float16)   # [64, T]
    sinh = np.sin(angle).T.astype(np.float16)
    cos2 = np.ascontiguousarray(np.vstack([cosh, cosh]))       # [128, T]
    sinpm = np.ascontiguousarray(np.vstack([-sinh, sinh]))     # [128, T]

    sidx = np.arange(128)[:, None]
    tidx = np.arange(128)[None, :]
    tri = np.ascontiguousarray((sidx <= tidx).astype(np.float16))
    ident = np.eye(128, dtype=np.float16)

    in_maps = []
    for c in range(N_CORES):
        in_maps.append({
            "xT_full": xT,  # replicated: every core gets the full activations
            "wq": np.ascontiguousarray(
                Wqp[:, c * REP:(c + 1) * REP].reshape(C, HD_SHARD)),
            "wk": np.ascontiguousarray(Wkp[:, c]),
            "wv": np.ascontiguousarray(Wv16[:, c]),
            "wo": np.ascontiguousarray(Wo16[:, c * CO_SHARD:(c + 1) * CO_SHARD]),
            "cos2": cos2,
            "sinpm": sinpm,
            "tri": tri,
            "idin": ident,
        })
    return in_maps


def _run(x, Wq, Wk, Wv, Wo, bo, trace=False, trace_cores=None):
    in_maps = _host_prep(x, Wq, Wk, Wv, Wo, bo)
    if "nc" not in _CACHE:
        _CACHE["nc"] = _build_nc()
    nc = _CACHE["nc"]
    r = bass_utils.run_bass_kernel_spmd(
        nc, in_maps, core_ids=list(range(N_CORES)),
        trace=trace, trace_cores=trace_cores,
    )
    # single-pass assembly: write each fp16 shard into a preallocated f32
    # buffer (implicit upcast), add bias in place
    out = np.empty((BT, C), dtype=np.float32)
    for c in range(N_CORES):
        out[:, c * CO_SHARD:(c + 1) * CO_SHARD] = r.results[c]["out"]
    out += np.asarray(bo, dtype=np.float32)[None, :]
    return out.reshape(B, T, C), r


def kernel(x, Wq, Wk, Wv, Wo, bo):
    out, _ = _run(x, Wq, Wk, Wv, Wo, bo, trace=False)
    return out

